# Initial kernel scaffold
#
"""Multi-head self-attention (B=4, T=2048, C=768, H=12) on 8 trn2 NeuronCores.

Sharding: core c -> batch b=c//2, head-group g=c%2 (6 heads each).
Each core computes its 6 heads' attention and a partial output projection
(contraction over its 384 ctx dims). Host sums the 2 partials per batch
and adds the bias.

Per-core kernel (all matmuls in float32r, 1 cycle/row on the PE):
  Xb[2048,768] -> X^T (PE transpose) -> qT,kT,vT[384,2048] projections
  v_aug[t][128, 2x65]: v rows with a ones column (softmax denominators
  come out of the ctx matmul for free).
  scores^T chunk = kT_chunk.T @ qT  -> exp on ACT (scale folded in)
  ctx^T[65,Tq]  += v_aug.T @ P^T    (row 64 = sum of exp)
  normalize: R = ones x recip(sums) (PE outer product), ctxT = ctx_u * R
  out[t] = sum_m ctxT[m].T @ Wo[m]  -> DMA out (partial, pre-bias)
"""
import sys
import os

sys.path.insert(0, "/opt/trn_rl_repo")

import numpy as np

P = 128
T = 2048
C = 768
HD = 384          # per-core head columns (6 heads x 64)
D = 64
NT = T // P       # 16 T chunks of 128
KC = C // P       # 6 contraction chunks for C
MC = HD // P      # 3 chunks of head dims
NH = 6            # heads per core
HALF = 1024       # T_q blocking for the attention inner loop
VW = 2 * D + 2    # 130: v_aug column block per T chunk (2 heads x 65)

_cache = {}


def _build():
    import concourse.bass as bass
    import concourse.bacc as bacc
    import concourse.mybir as mybir
    import concourse.tile as tile
    from concourse.masks import make_identity
    from contextlib import ExitStack

    F32 = mybir.dt.float32
    F32R = mybir.dt.float32r
    AF = mybir.ActivationFunctionType
    ALU = mybir.AluOpType

    nc = bacc.Bacc("TRN2", target_bir_lowering=False, debug=False)
    x = nc.dram_tensor("x", [T, C], F32, kind="ExternalInput").ap()
    wq = nc.dram_tensor("wq", [C, HD], F32, kind="ExternalInput").ap()
    wk = nc.dram_tensor("wk", [C, HD], F32, kind="ExternalInput").ap()
    wv = nc.dram_tensor("wv", [C, HD], F32, kind="ExternalInput").ap()
    wo = nc.dram_tensor("wo", [HD, C], F32, kind="ExternalInput").ap()
    out = nc.dram_tensor("out", [T, C], F32, kind="ExternalOutput").ap()

    with tile.TileContext(nc) as tc, ExitStack() as ctx:
        consts = ctx.enter_context(tc.tile_pool(name="consts", bufs=1))
        ident = consts.tile([P, P], F32)
        make_identity(nc, ident)
        ones_r = consts.tile([1, D], F32R)
        nc.vector.tensor_scalar(ones_r[:], ident[0:1, 0:D], 0.0, 1.0, ALU.mult, ALU.add)

        big = ctx.enter_context(tc.tile_pool(name="big", bufs=12))
        wrp = ctx.enter_context(tc.tile_pool(name="wrp", bufs=1))
        vap = ctx.enter_context(tc.tile_pool(name="vap", bufs=1))
        work = ctx.enter_context(tc.tile_pool(name="work", bufs=3))
        outp = ctx.enter_context(tc.tile_pool(name="outp", bufs=3))
        norm = ctx.enter_context(tc.tile_pool(name="norm", bufs=2))

        # ---- load weights, cast to f32r (rounding producer = DVE copy)
        w_r = {}
        with tc.tile_pool(name="wstage", bufs=2) as wstage:
            for nm, src in (("q", wq), ("k", wk), ("v", wv)):
                for kc in range(KC):
                    st = wstage.tile([P, HD], F32, name=f"wst_{nm}{kc}", tag="wst")
                    nc.sync.dma_start(st[:], src[P * kc:P * (kc + 1), :])
                    t_r = wrp.tile([P, HD], F32R, name=f"w_{nm}{kc}")
                    nc.vector.tensor_copy(t_r[:], st[:])
                    w_r[nm, kc] = t_r
            wo_r = []
            for m in range(MC):
                st = wstage.tile([P, C], F32, name=f"wst_o{m}", tag="wsto")
                nc.sync.dma_start(st[:], wo[P * m:P * (m + 1), :])
                t_r = wrp.tile([P, C], F32R, name=f"wo_{m}")
                nc.vector.tensor_copy(t_r[:], st[:])
                wo_r.append(t_r)

        # ---- X^T via PE transpose: xt[kc] = X[:, 128kc:+128].T  (f32r)
        xt = [big.tile([P, T], F32R, name=f"xt{kc}", tag="big2048") for kc in range(KC)]
        with tc.tile_pool(name="xrp", bufs=3) as xrp, \
             tc.tile_pool(name="tp1", bufs=2, space="PSUM") as tp1:
            for tq in range(NT // 4):      # groups of 4 T chunks
                for kc in range(KC):
                    tp = tp1.tile([P, 512], F32, name=f"tp_{tq}_{kc}", tag="tp")
                    for i in range(4):
                        t_i = 4 * tq + i
                        if kc == 0 and i == 0:
                            pass
                        xr = None
                    # transpose 4 chunks of the same kc from 4 xr tiles
                    for i in range(4):
                        t_i = 4 * tq + i
                        xr = _xr_tile(nc, tc, xrp, x, t_i)
                        nc.tensor.transpose(tp[:, P * i:P * (i + 1)], xr[:, P * kc:P * (kc + 1)], ident[:])
                    nc.vector.tensor_copy(xt[kc][:, 512 * tq:512 * (tq + 1)], tp[:])

        # (helper defined below is hoisted; see _xr_tile)

        # ---- projections: qT,kT (f32r) and vT (f32) in [hd, T] layout
        qT = [big.tile([P, T], F32R, name=f"qT{m}", tag="big2048") for m in range(MC)]
        kT = [big.tile([P, T], F32R, name=f"kT{m}", tag="big2048") for m in range(MC)]
        va = [vap.tile([P, NT * VW], F32R, name=f"va{m}") for m in range(MC)]

        with tc.tile_pool(name="psproj", bufs=4, space="PSUM") as psproj, \
             tc.tile_pool(name="tp2", bufs=2, space="PSUM") as tp2, \
             tc.tile_pool(name="vtp", bufs=1) as vtp:
            for nm, dest in (("q", qT), ("k", kT), ("v", None)):
                for m in range(MC):
                    pss = [psproj.tile([P, 512], F32, name=f"ps_{nm}{m}{n}", tag="ps")
                           for n in range(4)]
                    for kc in range(KC):
                        for n in range(4):
                            nc.tensor.matmul(
                                pss[n][:],
                                w_r[nm, kc][:, P * m:P * (m + 1)],
                                xt[kc][:, 512 * n:512 * (n + 1)],
                                start=(kc == 0), stop=(kc == KC - 1),
                            )
                    if dest is not None:
                        for n in range(4):
                            nc.vector.tensor_copy(dest[m][:, 512 * n:512 * (n + 1)], pss[n][:])
                    else:
                        # vT chunk (f32) -> transpose -> v_aug (f32r)
                        vT = vtp.tile([P, T], F32, name=f"vT{m}", tag="vT")
                        for n in range(4):
                            nc.scalar.activation(vT[:, 512 * n:512 * (n + 1)], pss[n][:], AF.Copy)
                        vav = va[m][:].rearrange("p (t g c) -> p t g c", t=NT, g=2)
                        for t_i in range(NT):
                            tp = tp2.tile([P, P], F32, name=f"vtp{m}_{t_i}", tag="vtp")
                            nc.tensor.transpose(tp[:], vT[:, P * t_i:P * (t_i + 1)], ident[:])
                            nc.vector.tensor_copy(
                                vav[:, t_i, :, 0:D],
                                tp[:].rearrange("p (g c) -> p g c", g=2),
                            )
                        # ones column (col 64 of each 65-block)
                        nc.vector.tensor_scalar(
                            vav[:, :, :, D:D + 1],
                            ident[:, 0:2 * NT].rearrange("p (t g) -> p t g", t=NT)[:, :, :, None],
                            0.0, 1.0, ALU.mult, ALU.add,
                        )

        # ---- attention
        ctxT = [big.tile([P, T], F32R, name=f"ctxT{m}", tag="big2048") for m in range(MC)]
        with tc.tile_pool(name="sps", bufs=2, space="PSUM") as spsp, \
             tc.tile_pool(name="cps", bufs=1, space="PSUM") as cpsp, \
             tc.tile_pool(name="rps", bufs=1, space="PSUM") as rpsp:
            for h in range(NH):
                m, par = divmod(h, 2)
                qh = qT[m][D * par:D * (par + 1), :]
                kh = kT[m][D * par:D * (par + 1), :]
                vav = va[m][:].rearrange("p (t g c) -> p t g c", t=NT, g=2)
                for half in range(T // HALF):
                    q0 = HALF * half
                    cps = cpsp.tile([D + 1, HALF], F32, name=f"cps{h}_{half}", tag="cps")
                    for j in range(NT):
                        sps = spsp.tile([P, HALF], F32, name=f"sps{h}{half}{j}", tag="sps")
                        for u in range(HALF // 512):
                            nc.tensor.matmul(
                                sps[:, 512 * u:512 * (u + 1)],
                                kh[:, P * j:P * (j + 1)],
                                qh[:, q0 + 512 * u:q0 + 512 * (u + 1)],
                                start=True, stop=True,
                            )
                        pt = work.tile([P, HALF], F32R, name=f"pt{h}{half}{j}", tag="pt")
                        nc.scalar.activation(pt[:], sps[:], AF.Exp, scale=float(D) ** -0.5)
                        for u in range(HALF // 512):
                            nc.tensor.matmul(
                                cps[:, 512 * u:512 * (u + 1)],
                                vav[:, j, par, :],
                                pt[:, 512 * u:512 * (u + 1)],
                                start=(j == 0), stop=(j == NT - 1),
                            )
                    # normalize this [64, HALF] ctx^T block
                    rr = norm.tile([1, HALF], F32, name=f"rr{h}{half}", tag="rr")
                    nc.vector.reciprocal_approx_fast(rr[:], cps[D:D + 1, :])
                    rr_r = norm.tile([1, HALF], F32R, name=f"rrr{h}{half}", tag="rrr")
                    nc.vector.tensor_copy(rr_r[:], rr[:])
                    ctx_u = norm.tile([D, HALF], F32, name=f"cu{h}{half}", tag="cu")
                    nc.scalar.activation(ctx_u[:], cps[0:D, :], AF.Copy)
                    for u in range(HALF // 512):
                        rps = rpsp.tile([D, 512], F32, name=f"rps{h}{half}{u}", tag="rps")
                        nc.tensor.matmul(rps[:], ones_r[:], rr_r[:, 512 * u:512 * (u + 1)],
                                         start=True, stop=True)
                        nc.vector.tensor_mul(
                            ctxT[m][D * par:D * (par + 1), q0 + 512 * u:q0 + 512 * (u + 1)],
                            ctx_u[:, 512 * u:512 * (u + 1)],
                            rps[:],
                        )

        # ---- output projection (partial over this core's 384 ctx dims)
        with tc.tile_pool(name="pso", bufs=2, space="PSUM") as psop:
            for t_i in range(NT):
                psA = psop.tile([P, 512], F32, name=f"psA{t_i}", tag="psA")
                psB = psop.tile([P, C - 512], F32, name=f"psB{t_i}", tag="psB")
                for m in range(MC):
                    nc.tensor.matmul(psA[:], ctxT[m][:, P * t_i:P * (t_i + 1)],
                                     wo_r[m][:, 0:512], start=(m == 0), stop=(m == MC - 1))
                    nc.tensor.matmul(psB[:], ctxT[m][:, P * t_i:P * (t_i + 1)],
                                     wo_r[m][:, 512:C], start=(m == 0), stop=(m == MC - 1))
                ob = outp.tile([P, C], F32, name=f"ob{t_i}", tag="ob")
                nc.scalar.activation(ob[:, 0:512], psA[:], AF.Copy)
                nc.scalar.activation(ob[:, 512:C], psB[:], AF.Copy)
                nc.sync.dma_start(out[P * t_i:P * (t_i + 1), :], ob[:])

    nc.compile()
    return nc


_XR = {}


def _xr_tile(nc, tc, xrp, x, t_i):
    """DMA-load X rows [128t:128t+128] once, reuse for all 6 kc transposes."""
    import concourse.mybir as mybir
    if t_i not in _XR:
        xr = xrp.tile([P, C], mybir.dt.float32, name=f"xr{t_i}", tag="xr")
        nc.sync.dma_start(xr[:], x[P * t_i:P * (t_i + 1), :])
        _XR[t_i] = xr
    return _XR[t_i]


def kernel(X, Wq, Wk, Wv, Wo, bo):
    from concourse import bass_utils

    if "nc" not in _cache:
        _XR.clear()
        _cache["nc"] = _build()
    nc = _cache["nc"]

    X = np.asarray(X, dtype=np.float32)
    in_maps = []
    for c in range(8):
        b, g = divmod(c, 2)
        sl = slice(HD * g, HD * (g + 1))
        in_maps.append({
            "x": np.ascontiguousarray(X[b]),
            "wq": np.ascontiguousarray(np.asarray(Wq, np.float32)[:, sl]),
            "wk": np.ascontiguousarray(np.asarray(Wk, np.float32)[:, sl]),
            "wv": np.ascontiguousarray(np.asarray(Wv, np.float32)[:, sl]),
            "wo": np.ascontiguousarray(np.asarray(Wo, np.float32)[sl, :]),
        })
    res = bass_utils.run_bass_kernel_spmd(nc, in_maps, core_ids=list(range(8)))
    B = X.shape[0] if X.ndim == 3 else 4
    outf = np.empty((4, T, C), np.float32)
    bo = np.asarray(bo, np.float32)
    for b in range(4):
        outf[b] = res.results[2 * b]["out"] + res.results[2 * b + 1]["out"] + bo
    return outf


# revision 13
# speedup vs baseline: 1.1820x; 1.1820x over previous
"""Multi-head self-attention (B=4, T=2048, C=768, H=12) on 8 trn2 NeuronCores.

Sharding: core c -> batch b=c//2, head-group g=c%2 (6 heads each).
Each core computes its 6 heads' attention and a partial output projection
(contraction over its 384 ctx dims). Host sums the 2 partials per batch
and adds the bias.

Per-core kernel (all matmuls in float32r, 1 cycle/row on the PE):
  Xb[2048,768] -> X^T (PE transpose) -> qT,kT,vT[384,2048] projections
  v_aug[t][128, 2x65]: v rows with a ones column (softmax denominators
  come out of the ctx matmul for free).
  scores^T chunk = kT_chunk.T @ qT  -> exp on ACT (scale folded in)
  ctx^T[65,Tq]  += v_aug.T @ P^T    (row 64 = sum of exp)
  normalize: R = ones x recip(sums) (PE outer product), ctxT = ctx_u * R
  out[t] = sum_m ctxT[m].T @ Wo[m]  -> DMA out (partial, pre-bias)

KERNEL_REPEAT=N builds the body N times (for overhead-cancelling timing).
"""
import sys
import os

sys.path.insert(0, "/opt/trn_rl_repo")

import numpy as np

P = 128
T = 2048
C = 768
HD = 384          # per-core head columns (6 heads x 64)
D = 64
NT = T // P       # 16 T chunks of 128
KC = C // P       # 6 contraction chunks for C
MC = HD // P      # 3 chunks of head dims
NH = 6            # heads per core
HALF = 1024       # T_q blocking for the attention inner loop
VW = 2 * D + 2    # 130: v_aug column block per T chunk (2 heads x 65)

_cache = {}


def _build(repeat=1):
    import concourse.bacc as bacc
    import concourse.mybir as mybir
    import concourse.tile as tile
    from concourse.masks import make_identity
    from contextlib import ExitStack

    F32 = mybir.dt.float32
    F32R = mybir.dt.float32r
    AF = mybir.ActivationFunctionType
    ALU = mybir.AluOpType

    nc = bacc.Bacc("TRN2", target_bir_lowering=False, debug=False)
    x = nc.dram_tensor("x", [T, C], F32, kind="ExternalInput").ap()
    wq = nc.dram_tensor("wq", [C, HD], F32, kind="ExternalInput").ap()
    wk = nc.dram_tensor("wk", [C, HD], F32, kind="ExternalInput").ap()
    wv = nc.dram_tensor("wv", [C, HD], F32, kind="ExternalInput").ap()
    wo = nc.dram_tensor("wo", [HD, C], F32, kind="ExternalInput").ap()
    out = nc.dram_tensor("out", [T, C], F32, kind="ExternalOutput").ap()

    def emit(pfx, tc, pools):
        (ident, ones_r), big, wrp, vap, work, outp, norm = pools

        # ---- load weights, cast to f32r (rounding producer = DVE copy)
        w_r = {}
        wo_r = []
        with tc.tile_pool(name=pfx + "wstage", bufs=2) as wstage:
            for nm, src in (("q", wq), ("k", wk), ("v", wv)):
                for kc in range(KC):
                    st = wstage.tile([P, HD], F32, name=f"{pfx}wst_{nm}{kc}", tag="wst")
                    nc.sync.dma_start(st[:], src[P * kc:P * (kc + 1), :])
                    t_r = wrp.tile([P, HD], F32R, name=f"{pfx}w_{nm}{kc}", tag=f"w_{nm}{kc}")
                    nc.vector.tensor_copy(t_r[:], st[:])
                    w_r[nm, kc] = t_r
            for m in range(MC):
                st = wstage.tile([P, C], F32, name=f"{pfx}wst_o{m}", tag="wsto")
                nc.sync.dma_start(st[:], wo[P * m:P * (m + 1), :])
                t_r = wrp.tile([P, C], F32R, name=f"{pfx}wo_{m}", tag=f"wo_{m}")
                nc.vector.tensor_copy(t_r[:], st[:])
                wo_r.append(t_r)

        # ---- X^T via PE transpose: xt[kc] = X[:, 128kc:+128].T  (f32r)
        xt = [big.tile([P, T], F32R, name=f"{pfx}xt{kc}", tag="big2048") for kc in range(KC)]
        with tc.tile_pool(name=pfx + "xrp", bufs=5) as xrp, \
             tc.tile_pool(name=pfx + "tp1", bufs=2, space="PSUM") as tp1:
            for tq in range(NT // 4):      # groups of 4 T chunks
                xrs = []
                for i in range(4):
                    t_i = 4 * tq + i
                    xr = xrp.tile([P, C], F32, name=f"{pfx}xr{t_i}", tag="xr")
                    nc.sync.dma_start(xr[:], x[P * t_i:P * (t_i + 1), :])
                    xrs.append(xr)
                for kc in range(KC):
                    tp = tp1.tile([P, 512], F32, name=f"{pfx}tp_{tq}_{kc}", tag="tp")
                    for i in range(4):
                        nc.tensor.transpose(tp[:, P * i:P * (i + 1)], xrs[i][:, P * kc:P * (kc + 1)], ident[:])
                    nc.vector.tensor_copy(xt[kc][:, 512 * tq:512 * (tq + 1)], tp[:])

        # ---- projections: qT,kT (f32r) and v -> v_aug
        qT = [big.tile([P, T], F32R, name=f"{pfx}qT{m}", tag="big2048") for m in range(MC)]
        kT = [big.tile([P, T], F32R, name=f"{pfx}kT{m}", tag="big2048") for m in range(MC)]
        va = [vap.tile([P, NT * VW], F32R, name=f"{pfx}va{m}", tag=f"va{m}") for m in range(MC)]

        with tc.tile_pool(name=pfx + "psproj", bufs=4, space="PSUM") as psproj, \
             tc.tile_pool(name=pfx + "tp2", bufs=2, space="PSUM") as tp2, \
             tc.tile_pool(name=pfx + "vtp", bufs=1) as vtp:
            for nm, dest in (("q", qT), ("k", kT), ("v", None)):
                for m in range(MC):
                    pss = [psproj.tile([P, 512], F32, name=f"{pfx}ps_{nm}{m}{n}", tag="ps")
                           for n in range(4)]
                    for kc in range(KC):
                        for n in range(4):
                            nc.tensor.matmul(
                                pss[n][:],
                                w_r[nm, kc][:, P * m:P * (m + 1)],
                                xt[kc][:, 512 * n:512 * (n + 1)],
                                start=(kc == 0), stop=(kc == KC - 1),
                            )
                    if dest is not None:
                        for n in range(4):
                            nc.vector.tensor_copy(dest[m][:, 512 * n:512 * (n + 1)], pss[n][:])
                    else:
                        # vT chunk (f32) -> transpose -> v_aug (f32r)
                        vT = vtp.tile([P, T], F32, name=f"{pfx}vT{m}", tag="vT")
                        for n in range(4):
                            nc.scalar.activation(vT[:, 512 * n:512 * (n + 1)], pss[n][:], AF.Copy)
                        vav = va[m][:].rearrange("p (t g c) -> p t g c", t=NT, g=2)
                        for t_i in range(NT):
                            tp = tp2.tile([P, P], F32, name=f"{pfx}vtp{m}_{t_i}", tag="vtp")
                            nc.tensor.transpose(tp[:], vT[:, P * t_i:P * (t_i + 1)], ident[:])
                            nc.vector.tensor_copy(
                                vav[:, t_i, :, 0:D],
                                tp[:].rearrange("p (g c) -> p g c", g=2),
                            )
                        # ones column (col 64 of each 65-block)
                        nc.vector.tensor_scalar(
                            vav[:, :, :, D:D + 1],
                            ident[:, 0:2 * NT].rearrange("p (t g c) -> p t g c", t=NT, g=2, c=1),
                            0.0, 1.0, ALU.mult, ALU.add,
                        )

        # ---- attention
        ctxT = [big.tile([P, T], F32R, name=f"{pfx}ctxT{m}", tag="big2048") for m in range(MC)]
        with tc.tile_pool(name=pfx + "sps", bufs=2, space="PSUM") as spsp, \
             tc.tile_pool(name=pfx + "cps", bufs=1, space="PSUM") as cpsp, \
             tc.tile_pool(name=pfx + "rps", bufs=1, space="PSUM") as rpsp:
            for h in range(NH):
                m, par = divmod(h, 2)
                qh = qT[m][D * par:D * (par + 1), :]
                kh = kT[m][D * par:D * (par + 1), :]
                vav = va[m][:].rearrange("p (t g c) -> p t g c", t=NT, g=2)
                for half in range(T // HALF):
                    q0 = HALF * half
                    cps = cpsp.tile([D + 1, HALF], F32, name=f"{pfx}cps{h}_{half}", tag="cps")
                    for j in range(NT):
                        sps = spsp.tile([P, HALF], F32, name=f"{pfx}sps{h}{half}{j}", tag="sps")
                        for u in range(HALF // 512):
                            nc.tensor.matmul(
                                sps[:, 512 * u:512 * (u + 1)],
                                kh[:, P * j:P * (j + 1)],
                                qh[:, q0 + 512 * u:q0 + 512 * (u + 1)],
                                start=True, stop=True,
                            )
                        pt = work.tile([P, HALF], F32R, name=f"{pfx}pt{h}{half}{j}", tag="pt")
                        nc.scalar.activation(pt[:], sps[:], AF.Exp, scale=float(D) ** -0.5)
                        for u in range(HALF // 512):
                            nc.tensor.matmul(
                                cps[:, 512 * u:512 * (u + 1)],
                                vav[:, j, par, :],
                                pt[:, 512 * u:512 * (u + 1)],
                                start=(j == 0), stop=(j == NT - 1),
                            )
                    # normalize this [64, HALF] ctx^T block
                    s_sb = norm.tile([1, HALF], F32, name=f"{pfx}ssb{h}{half}", tag="ssb")
                    nc.vector.tensor_copy(s_sb[:], cps[D:D + 1, :])
                    rr = norm.tile([1, HALF], F32, name=f"{pfx}rr{h}{half}", tag="rr")
                    nc.vector.reciprocal_approx_fast(rr[:], s_sb[:])
                    rr_r = norm.tile([1, HALF], F32R, name=f"{pfx}rrr{h}{half}", tag="rrr")
                    nc.vector.tensor_copy(rr_r[:], rr[:])
                    ctx_u = norm.tile([D, HALF], F32, name=f"{pfx}cu{h}{half}", tag="cu")
                    nc.scalar.activation(ctx_u[:], cps[0:D, :], AF.Copy)
                    for u in range(HALF // 512):
                        rps = rpsp.tile([D, 512], F32, name=f"{pfx}rps{h}{half}{u}", tag="rps")
                        nc.tensor.matmul(rps[:], ones_r[:], rr_r[:, 512 * u:512 * (u + 1)],
                                         start=True, stop=True)
                        nc.vector.tensor_mul(
                            ctxT[m][D * par:D * (par + 1), q0 + 512 * u:q0 + 512 * (u + 1)],
                            ctx_u[:, 512 * u:512 * (u + 1)],
                            rps[:],
                        )

        # ---- output projection (partial over this core's 384 ctx dims)
        with tc.tile_pool(name=pfx + "pso", bufs=2, space="PSUM") as psop:
            for t_i in range(NT):
                psA = psop.tile([P, 512], F32, name=f"{pfx}psA{t_i}", tag="psA")
                psB = psop.tile([P, C - 512], F32, name=f"{pfx}psB{t_i}", tag="psB")
                for m in range(MC):
                    nc.tensor.matmul(psA[:], ctxT[m][:, P * t_i:P * (t_i + 1)],
                                     wo_r[m][:, 0:512], start=(m == 0), stop=(m == MC - 1))
                    nc.tensor.matmul(psB[:], ctxT[m][:, P * t_i:P * (t_i + 1)],
                                     wo_r[m][:, 512:C], start=(m == 0), stop=(m == MC - 1))
                ob = outp.tile([P, C], F32, name=f"{pfx}ob{t_i}", tag="ob")
                nc.scalar.activation(ob[:, 0:512], psA[:], AF.Copy)
                nc.scalar.activation(ob[:, 512:C], psB[:], AF.Copy)
                nc.sync.dma_start(out[P * t_i:P * (t_i + 1), :], ob[:])

    with tile.TileContext(nc) as tc, ExitStack() as ctx:
        consts = ctx.enter_context(tc.tile_pool(name="consts", bufs=1))
        ident = consts.tile([P, P], F32)
        make_identity(nc, ident)
        ones_r = consts.tile([1, D], F32R)
        nc.vector.tensor_scalar(ones_r[:], ident[0:1, 0:D], 0.0, 1.0, ALU.mult, ALU.add)

        big = ctx.enter_context(tc.tile_pool(name="big", bufs=12))
        wrp = ctx.enter_context(tc.tile_pool(name="wrp", bufs=1))
        vap = ctx.enter_context(tc.tile_pool(name="vap", bufs=1))
        work = ctx.enter_context(tc.tile_pool(name="work", bufs=2))
        outp = ctx.enter_context(tc.tile_pool(name="outp", bufs=2))
        norm = ctx.enter_context(tc.tile_pool(name="norm", bufs=1))
        pools = ((ident, ones_r), big, wrp, vap, work, outp, norm)
        for rep in range(repeat):
            emit(f"r{rep}_", tc, pools)

    nc.compile()
    return nc


def kernel(X, Wq, Wk, Wv, Wo, bo):
    from concourse import bass_utils

    if "nc" not in _cache:
        _cache["nc"] = _build(int(os.environ.get("KERNEL_REPEAT", "1")))
    nc = _cache["nc"]

    X = np.asarray(X, dtype=np.float32)
    in_maps = []
    for c in range(8):
        b, g = divmod(c, 2)
        sl = slice(HD * g, HD * (g + 1))
        in_maps.append({
            "x": np.ascontiguousarray(X[b]),
            "wq": np.ascontiguousarray(np.asarray(Wq, np.float32)[:, sl]),
            "wk": np.ascontiguousarray(np.asarray(Wk, np.float32)[:, sl]),
            "wv": np.ascontiguousarray(np.asarray(Wv, np.float32)[:, sl]),
            "wo": np.ascontiguousarray(np.asarray(Wo, np.float32)[sl, :]),
        })
    res = bass_utils.run_bass_kernel_spmd(nc, in_maps, core_ids=list(range(8)))
    outf = np.empty((4, T, C), np.float32)
    bo = np.asarray(bo, np.float32)
    for b in range(4):
        outf[b] = res.results[2 * b]["out"] + res.results[2 * b + 1]["out"] + bo
    return outf
